# revision 28
# baseline (speedup 1.0000x reference)
"""Trainium2 Bass kernel for nn_BayesianOddLayer (GNN message passing).

Computation (per reference):
    total_mask = w_odd2even_mask * odd_weights              # [E, E]
    z          = (u < sigmoid(dropout_logits))              # [E]
    msg        = x @ (total_mask * z[:, None])              # [B, E]
    skip       = llr @ (w_skipconn2even_mask * llr_weights) # [B, E]
    out        = tanh(0.5 * clip(msg + skip, -10, 10))

Structure exploited: w_odd2even_mask[e1, e2] is nonzero only when
var(e1) == var(e2) (Tanner graph), and the skip term feeds each edge
from exactly its own variable.  The 512 variables are packed into 16
OUTPUT TILES of exactly 128 edges each (whole variables per tile), and
the tiles' variables into 4 VAR-TILES of <= 128 variables.  Each output
tile is then TWO accumulating matmuls into the same PSUM region:
    skip: lhsT = masked lw block          [128 var-tile vars, 128 tile edges]
    msg : lhsT = masked/z-gated ow block  [128 tile edges, 128 tile edges]
The z-independent skip matmuls run FIRST (start=True) so the dropout
chain (u < sigmoid(logits), DVE+ACT) and the per-tile z-gating stay off
the ramp critical path; the msg matmuls accumulate behind them.

Engine budget per core: ACT tanh is the floor (16 groups x 2048
elems/lane at 1/cycle @1.2GHz + 352c pipeline fill ~ 32us) with matmuls
(~1.7us/group at 2.4GHz) hidden under it.  The combined DMA fabric
(~400 GB/s, and HALF that until the HAM clock gate releases ~4.5us
after first PE activity) binds the ramp, so the moving x operand and
the 0/1 masks ship as fp8 E3M4 (exact for masks; |x|<=5.4 < 15.5 max
normal, host-verified max output err 1.3e-2 vs the 2e-2 gate) while
weights/llr stay fp16 (fp8 weights would add another ~1.1e-2).  Mixed
fp16-lhsT x fp8-rhs matmuls are architecturally fine (both upcast to
fp22).  Output int8 (t*127 on DVE, |err| <= 0.004), halving stores.
Weight quads ride FOUR per-quad DMAs interleaved with the first rhs
group loads so quad q's prep completes just before group q's msg
matmuls; warmup matmuls bridge PE activity to the first real matmul
(an idle PE gap >~2us re-gates the clock to 1.2GHz).  Group 0's tanh
runs as two 1024-col halves so ACT starts ~1us earlier.  Stores:
chunks 0-1 as single 1MB SWDGE DMAs, chunk 2 on the by-then-idle sync
ring, last chunk per-group on alternating rings.

Sharding: data-parallel over batch across 8 NeuronCores; weights
replicated.
"""

from contextlib import ExitStack

import numpy as np
import ml_dtypes

import concourse.bass as bass
import concourse.mybir as mybir
from concourse import bacc
from concourse.bass_utils import run_bass_kernel_spmd
from concourse.tile import TileContext

F32 = mybir.dt.float32
F16 = mybir.dt.float16
F8 = mybir.dt.float8e3
I8 = mybir.dt.int8
U8 = mybir.dt.uint8
AF = mybir.ActivationFunctionType
ALU = mybir.AluOpType

E3M4 = ml_dtypes.float8_e3m4

B = 16384  # batch
E = 2048  # edges
NV = 512  # variable nodes
NCORES = 8
BSH = B // NCORES  # batch rows per core
CHUNK = 512  # batch columns per matmul (hw limit on the moving operand)
NCHUNK = BSH // CHUNK
P = 128  # partitions
NT = E // P  # output tiles (16), each exactly 128 edges
NQ = NT // 4  # quads = ACT groups per chunk (4)
NWARM = 14  # PE warmup matmuls (keep PE busy HAM-release -> first real matmul)
QSCALE = 127.0  # int8 output quantization scale

# byte-packed layouts (per partition row)
UHDB = 4 * NT * 2  # u/lg header bytes: NT f32 u + NT f32 lg = 128
WQB = 4 * P * 2 + 4 * P + 4 * P + 4 * P * 2  # quad: ew f16|em f8|sm f8|sw f16
EWO, EMO, SMO, SWO = 0, 4 * P * 2, 4 * P * 2 + 4 * P, 4 * P * 2 + 8 * P
GWB = CHUNK * 2 + 4 * CHUNK  # rhs (chunk, group) bytes: vt f16 | rt f8
W = NT * CHUNK  # out free-dim per chunk


def _plan_tiles(w_skipconn2even_mask: np.ndarray):
    """Pack whole variables into NT tiles of exactly P edges each, and the
    tiles' variables into NQ var-tiles of <= P variables (tile t's vars
    live in var-tile t//4).

    Returns (tile_edges [NT][P], vtile_vars [NQ][<=P]).
    """
    var = w_skipconn2even_mask.argmax(axis=0).astype(np.int64)  # [E]
    deg = np.bincount(var, minlength=NV)
    vars_nz = np.where(deg > 0)[0]
    order = vars_nz[np.argsort(-deg[vars_nz], kind="stable")]
    gsum = np.zeros(NT, np.int64)
    gnv = np.zeros(NT, np.int64)
    groups = [[] for _ in range(NT)]
    for v in order:
        dv = int(deg[v])
        cand = [g for g in range(NT) if gsum[g] + dv <= P]
        assert cand, "greedy packing failed"
        g = min(cand, key=lambda g: (gsum[g], gnv[g]))
        groups[g].append(int(v))
        gsum[g] += dv
        gnv[g] += 1
    assert all(s == P for s in gsum), f"imperfect packing {gsum}"

    # assign the 16 groups to 4 var-tiles (4 each), balancing #vars <= P
    tile_nv = np.zeros(NQ, np.int64)
    tile_cnt = np.zeros(NQ, np.int64)
    assign = [[] for _ in range(NQ)]
    for g in np.argsort(-gnv, kind="stable"):
        q = min(
            [q for q in range(NQ) if tile_cnt[q] < 4], key=lambda q: tile_nv[q]
        )
        assign[q].append(int(g))
        tile_nv[q] += gnv[g]
        tile_cnt[q] += 1
    assert all(n <= P for n in tile_nv), f"var-tile overflow {tile_nv}"

    edges_of = {v: np.where(var == v)[0] for v in vars_nz}
    tile_edges = []
    vtile_vars = []
    for q in range(NQ):
        vlist = []
        for g in assign[q]:
            gv = sorted(groups[g])
            vlist.extend(gv)
            te = np.concatenate([edges_of[v] for v in gv])
            assert te.size == P
            tile_edges.append(te)
        vtile_vars.append(np.array(vlist))
    assert sum(t.size for t in tile_edges) == E
    return tile_edges, vtile_vars


def _build_nc(need_clamp):
    nc = bacc.Bacc("TRN2", target_bir_lowering=False, debug=False,
                   num_devices=NCORES)
    wcomb = nc.dram_tensor(
        "wcomb", [P, UHDB + NQ * WQB], U8, kind="ExternalInput").ap()
    rhsp = nc.dram_tensor(
        "rhsp", [P, NCHUNK * NQ * GWB], U8, kind="ExternalInput").ap()
    outp = nc.dram_tensor("outp", [P, NCHUNK * W], I8, kind="ExternalOutput").ap()
    # the very last group stores fp16 directly (its int8 convert would sit
    # on the critical tail after the final tanh)
    outp16 = nc.dram_tensor("outp16", [P, 4 * CHUNK], F16, kind="ExternalOutput").ap()

    with TileContext(nc) as tc, ExitStack() as ctx:
        cpool = ctx.enter_context(tc.tile_pool(name="const", bufs=1))
        r0pool = ctx.enter_context(tc.tile_pool(name="rhs0", bufs=2))
        rcpool = ctx.enter_context(tc.tile_pool(name="rhsc", bufs=2))
        opool = ctx.enter_context(tc.tile_pool(name="out", bufs=4))
        o8pool = ctx.enter_context(tc.tile_pool(name="out8", bufs=4))
        pspool = ctx.enter_context(tc.tile_pool(name="ps", bufs=2, space="PSUM"))

        # PE warmup operands: memset first on gpsimd so warmups run during
        # the initial DMA window and keep the HAM clock gate open (HAM
        # re-evaluates every ~3.4us window on PE busy fraction)
        zl = cpool.tile([P, P], F16)
        nc.gpsimd.memset(zl[:], 0.0)
        zr = cpool.tile([P, CHUNK], F16)
        nc.gpsimd.memset(zr[:], 0.0)

        # u/lg rides its own tiny first DMA (small-transfer receipt fires
        # earliest); then quads 0-1 as ONE fat DMA (per-transfer fixed
        # latency ~2us makes many small DMAs strictly worse).  Quads 2-3
        # go right behind the first rhs load (separate tile: a shared tile
        # would add WAR hazards with quads 0-1 matmul reads).
        ut = cpool.tile([P, UHDB], U8)
        nc.sync.dma_start(ut[:], wcomb[:, 0:UHDB])
        wA = cpool.tile([P, 2 * WQB], U8)
        nc.sync.dma_start(wA[:, 0:WQB], wcomb[:, UHDB : UHDB + WQB])
        wB = cpool.tile([P, 2 * WQB], U8)

        def wq(q):
            t = wA if q < 2 else wB
            return t[:, (q % 2) * WQB : (q % 2 + 1) * WQB]

        def ew(q, i):  # [P tile edges, P out edges] f16
            w = wq(q)
            return w[:, EWO + i * 2 * P : EWO + (i + 1) * 2 * P].bitcast(F16)

        def em(q, i):  # 0/1 mask, fp8 (exact)
            w = wq(q)
            return w[:, EMO + i * P : EMO + (i + 1) * P].bitcast(F8)

        def sw(q, i):  # [P vars, P out edges] f16
            w = wq(q)
            return w[:, SWO + i * 2 * P : SWO + (i + 1) * 2 * P].bitcast(F16)

        # z = (u < sigmoid(dropout_logits)) in fp32 (u/logits arrive as raw
        # fp32 bytes; DVE copy feeds ACT a clean f32 tile since ACT cannot
        # take bitcast APs).  (Measured: a broadcast z-block tile + 512-col
        # TT prep, with or without high_priority pinning, reshuffles the
        # early schedule and LOSES 3-4us -- keep the per-tile STT form.)
        zt = cpool.tile([P, NT], F32)
        nc.vector.tensor_copy(zt[:], ut[:, 4 * NT : 8 * NT].bitcast(F32))
        nc.scalar.activation(zt[:], zt[:], AF.Sigmoid)
        nc.vector.tensor_tensor(
            zt[:], ut[:, 0 : 4 * NT].bitcast(F32), zt[:], ALU.is_lt)

        wps = pspool.tile([P, 4 * CHUNK], F32, tag="ps")
        for _ in range(NWARM):
            nc.tensor.matmul(wps[:, 0:CHUNK], zl[:], zr[:], start=True, stop=True)

        # weight prep (all DVE; gpsimd elementwise is 35x slower): skip
        # block first (sw *= sm, one 512-col op, unblocks the z-independent
        # skip matmuls), then (ew * z[tile]) * em fused per tile
        def prep_quad(q):
            w = wq(q)
            swb = w[:, SWO : SWO + 8 * P].bitcast(F16)
            smb = w[:, SMO : SMO + 4 * P].bitcast(F8)
            nc.vector.tensor_tensor(swb, swb, smb, ALU.mult)
            for i in range(4):
                t = 4 * q + i
                nc.vector.scalar_tensor_tensor(
                    ew(q, i), ew(q, i), zt[:, t : t + 1], em(q, i),
                    ALU.mult, ALU.mult)

        prep_quad(0)

        for nb in range(NCHUNK):
            for q in range(NQ):
                # rhs loads: chunks 0-1 as per-group DMAs with the quad-2/3
                # weight transfers interleaved so every group's operands
                # land just before its matmuls (these all transfer after
                # the HAM clock release, where per-transfer overhead is
                # small); chunks 2-3 as single whole-chunk DMAs into
                # separate buffers (no WAR gating, issued back to back)
                if nb <= 1:
                    if nb == 0 and q == 0:
                        # (measured: moving these to the gpsimd ring to run
                        # both rings during the ramp THROTTLES the fabric,
                        # +1.0-1.5us stalls at T2-T4 -- keep one ring)
                        r01 = []
                        for g in range(2 * NQ):
                            rg = r0pool.tile([P, GWB], U8, name=f"rg{g}",
                                             bufs=1)
                            r01.append(rg)
                            nc.sync.dma_start(
                                rg[:], rhsp[:, g * GWB : (g + 1) * GWB])
                            if g == 0:
                                nc.sync.dma_start(
                                    wA[:, WQB:],
                                    wcomb[:, UHDB + WQB : UHDB + 2 * WQB])
                                prep_quad(1)
                            if g == 1:
                                nc.sync.dma_start(
                                    wB[:, 0:WQB],
                                    wcomb[:, UHDB + 2 * WQB : UHDB + 3 * WQB])
                                prep_quad(2)
                            if g == 2:
                                nc.sync.dma_start(
                                    wB[:, WQB:],
                                    wcomb[:, UHDB + 3 * WQB : UHDB + 4 * WQB])
                                prep_quad(3)
                    rtv = r01[nb * NQ + q][:]
                else:
                    if q == 0:
                        rc = rcpool.tile([P, NQ * GWB], U8, name="rc")
                        c0 = nb * NQ * GWB
                        nc.sync.dma_start(rc[:], rhsp[:, c0 : c0 + NQ * GWB])
                    rtv = rc[:, q * GWB : (q + 1) * GWB]
                vt = rtv[:, 0 : 2 * CHUNK].bitcast(F16)

                def rt(i):
                    o = 2 * CHUNK + i * CHUNK
                    return rtv[:, o : o + CHUNK].bitcast(F8)

                ps = pspool.tile([P, 4 * CHUNK], F32)
                # z-independent skip matmuls first, then the accumulating
                # z-gated msg matmuls.  Group (0,0) interleaves per HALF
                # (skip01, msg01, skip23, msg23 -- no same-bank neighbors)
                # so its first tanh half fires after 4 matmuls, ~0.9us
                # earlier.
                halves = [(0, 1), (2, 3)] if nb == 0 and q == 0 else [(0, 1, 2, 3)]
                for idx in halves:
                    for i in idx:
                        nc.tensor.matmul(
                            ps[:, i * CHUNK : (i + 1) * CHUNK],
                            sw(q, i), vt, start=True, stop=False)
                    for i in idx:
                        nc.tensor.matmul(
                            ps[:, i * CHUNK : (i + 1) * CHUNK],
                            ew(q, i), rt(i), start=False, stop=True)
                last = nb == NCHUNK - 1 and q == NQ - 1
                ot = opool.tile([P, 4 * CHUNK], F16)
                if need_clamp:
                    nc.vector.tensor_scalar(
                        ot[:], ps[:], 10.0, -10.0, ALU.min, ALU.max)
                    nc.scalar.activation(ot[:], ot[:], AF.Tanh, scale=0.5)
                elif nb == 0 and q == 0:
                    # first group in two halves: ACT starts ~1us earlier
                    h = 2 * CHUNK
                    nc.scalar.activation(
                        ot[:, 0:h], ps[:, 0:h], AF.Tanh, scale=0.5)
                    nc.scalar.activation(
                        ot[:, h:], ps[:, h:], AF.Tanh, scale=0.5)
                elif last:
                    # final group in two halves: the first half's store
                    # drains (gpsimd ring, so it never queues ahead of the
                    # very last sync-ring store) while the second half's
                    # tanh still runs
                    h = 2 * CHUNK
                    nc.scalar.activation(
                        ot[:, 0:h], ps[:, 0:h], AF.Tanh, scale=0.5)
                    nc.gpsimd.dma_start(outp16[:, 0:h], ot[:, 0:h])
                    nc.scalar.activation(
                        ot[:, h:], ps[:, h:], AF.Tanh, scale=0.5)
                else:
                    # clip(v, +-10) proven identity for these inputs (host
                    # bound); tanh straight from PSUM
                    nc.scalar.activation(ot[:], ps[:], AF.Tanh, scale=0.5)
                # int8 quantize on DVE (round-to-nearest, ~1.2us per group)
                if nb < NCHUNK - 2:
                    # chunks 0-1: quantize into a per-chunk tile, store as
                    # ONE 1MB DMA (small SWDGE stores sustain only ~110GB/s
                    # under load contention, and these overlap the loads)
                    if q == 0:
                        o8c = o8pool.tile([P, NT * CHUNK], I8, tag="o8c", bufs=2)
                    nc.vector.tensor_scalar(
                        o8c[:, q * 4 * CHUNK : (q + 1) * 4 * CHUNK],
                        ot[:], QSCALE, None, ALU.mult)
                    if q == NQ - 1:
                        nc.gpsimd.dma_start(outp[:, nb * W : (nb + 1) * W], o8c[:])
                elif nb == NCHUNK - 2:
                    # chunk 2: loads are done by now -- store per-group on
                    # the gpsimd ring so the drain spreads out instead of
                    # bunching a 1MB store into the tail
                    o8 = o8pool.tile([P, 4 * CHUNK], I8)
                    nc.vector.tensor_scalar(o8[:], ot[:], QSCALE, None, ALU.mult)
                    c0 = nb * W + q * 4 * CHUNK
                    nc.gpsimd.dma_start(outp[:, c0 : c0 + 4 * CHUNK], o8[:])
                elif q < NQ - 1:
                    # last chunk: per-group stores alternating rings so the
                    # final drain is parallel and fine-grained
                    o8 = o8pool.tile([P, 4 * CHUNK], I8)
                    nc.vector.tensor_scalar(o8[:], ot[:], QSCALE, None, ALU.mult)
                    c0 = nb * W + q * 4 * CHUNK
                    if q % 2 == 0:
                        nc.sync.dma_start(outp[:, c0 : c0 + 4 * CHUNK], o8[:])
                    else:
                        nc.gpsimd.dma_start(outp[:, c0 : c0 + 4 * CHUNK], o8[:])
                elif need_clamp:
                    nc.sync.dma_start(outp16[:], ot[:])
                else:
                    # final group: fp16 straight out, no convert on the
                    # tail (first half's store was issued above); the very
                    # last store is split across BOTH rings -- per-transfer
                    # latency (~2us) dominates 256KB stores, so two
                    # parallel 128KB transfers finish ~1us sooner
                    nc.sync.dma_start(
                        outp16[:, 2 * CHUNK : 3 * CHUNK],
                        ot[:, 2 * CHUNK : 3 * CHUNK])
                    nc.gpsimd.dma_start(
                        outp16[:, 3 * CHUNK :], ot[:, 3 * CHUNK :])
    nc.compile()
    return nc


def _prep(x, llr, u, odd_weights, llr_weights, dropout_logits,
          w_odd2even_mask, w_skipconn2even_mask):
    """Host-side data movement: tile packing, block gathers, shards, casts."""
    ow = np.asarray(odd_weights, np.float32)
    msk = np.asarray(w_odd2even_mask, np.float32)
    lw = np.asarray(llr_weights, np.float32)
    smask = np.asarray(w_skipconn2even_mask, np.float32)
    u = np.asarray(u, np.float32)
    lg = np.asarray(dropout_logits, np.float32)

    tile_edges, vtile_vars = _plan_tiles(smask)

    wblk = np.zeros((P, NQ * WQB), np.uint8)
    ucomb = np.zeros((P, NT), np.float32)
    lgcomb = np.zeros((P, NT), np.float32)
    for t in range(NT):
        q = t // 4
        i = t % 4
        pe = tile_edges[t]
        vs = vtile_vars[q]
        c = q * WQB
        wblk[:, c + EWO + i * 2 * P : c + EWO + (i + 1) * 2 * P] = (
            ow[np.ix_(pe, pe)].astype(np.float16).view(np.uint8))
        wblk[:, c + EMO + i * P : c + EMO + (i + 1) * P] = (
            msk[np.ix_(pe, pe)].astype(E3M4).view(np.uint8))
        wblk[: vs.size, c + SWO + i * 2 * P : c + SWO + (i + 1) * 2 * P] = (
            lw[np.ix_(vs, pe)].astype(np.float16).view(np.uint8))
        wblk[: vs.size, c + SMO + i * P : c + SMO + (i + 1) * P] = (
            smask[np.ix_(vs, pe)].astype(E3M4).view(np.uint8))
        ucomb[:, t] = u[pe]
        lgcomb[:, t] = lg[pe]

    x = np.asarray(x, np.float32)
    llr = np.asarray(llr, np.float32)

    # Rigorous bound on |msg + skip| with the exact shipped (quantized)
    # values: if it cannot reach the +-10 clip, the clip is the identity
    # and the device clamp stage is elided.
    xmax = float(np.abs(x.astype(E3M4).astype(np.float32)).max())
    lmax = float(np.abs(llr.astype(np.float16).astype(np.float32)).max())
    awe = np.zeros((NT, P), np.float64)  # sum_e' |ow*mask| per out edge
    aws = np.zeros((NT, P), np.float64)
    for t in range(NT):
        q = t // 4
        c = q * WQB
        ewf = wblk[:, c + EWO + t % 4 * 2 * P : c + EWO + (t % 4 + 1) * 2 * P
                   ].view(np.float16).astype(np.float64)
        emf = wblk[:, c + EMO + t % 4 * P : c + EMO + (t % 4 + 1) * P
                   ].view(E3M4).astype(np.float64)
        swf = wblk[:, c + SWO + t % 4 * 2 * P : c + SWO + (t % 4 + 1) * 2 * P
                   ].view(np.float16).astype(np.float64)
        smf = wblk[:, c + SMO + t % 4 * P : c + SMO + (t % 4 + 1) * P
                   ].view(E3M4).astype(np.float64)
        awe[t] = np.abs(ewf * emf).sum(axis=0)
        aws[t] = np.abs(swf * smf).sum(axis=0)
    bound = float((awe * xmax + aws * lmax).max())
    need_clamp = bound >= 9.5

    in_maps = []
    for c in range(NCORES):
        sl = slice(c * BSH, (c + 1) * BSH)
        xq = np.ascontiguousarray(x[sl].T).astype(E3M4).view(np.uint8)  # [E, BSH]
        lq = np.ascontiguousarray(llr[sl].T).astype(np.float16)  # [NV, BSH]
        rhsp = np.zeros((P, NCHUNK, NQ, GWB), np.uint8)
        for q in range(NQ):
            vs = vtile_vars[q]
            vtb = lq[vs].view(np.uint8).reshape(vs.size, NCHUNK, 2 * CHUNK)
            rhsp[: vs.size, :, q, 0 : 2 * CHUNK] = vtb.transpose(0, 1, 2)
            for i in range(4):
                t = 4 * q + i
                rtb = xq[tile_edges[t]].reshape(P, NCHUNK, CHUNK)
                o = 2 * CHUNK + i * CHUNK
                rhsp[:, :, q, o : o + CHUNK] = rtb
        in_maps.append({
            "wcomb": np.ascontiguousarray(np.concatenate(
                [ucomb.view(np.uint8), lgcomb.view(np.uint8), wblk], axis=1)),
            "rhsp": np.ascontiguousarray(rhsp.reshape(P, -1)),
        })
    return tile_edges, in_maps, need_clamp


def _run(inputs: dict, trace: bool = False, **kwargs):
    tile_edges, in_maps, need_clamp = _prep(**inputs)
    nc = _build_nc(need_clamp)
    res = run_bass_kernel_spmd(nc, in_maps, list(range(NCORES)), trace=trace, **kwargs)

    dest = np.concatenate(tile_edges)  # row (t, p) -> edge column
    out = np.empty((B, E), np.float32)
    for c in range(NCORES):
        sl = slice(c * BSH, (c + 1) * BSH)
        a8 = (res.results[c]["outp"]
              .reshape(P, NCHUNK, NT, CHUNK)
              .astype(np.float32) * np.float32(1.0 / QSCALE))
        # last group (chunk 3, tiles 12-15) arrived as raw fp16
        a16 = (res.results[c]["outp16"]
               .reshape(P, 4, CHUNK)
               .astype(np.float32))
        a8[:, NCHUNK - 1, NT - 4 : NT, :] = a16
        arr = a8.transpose(2, 0, 1, 3).reshape(NT * P, BSH)
        out[sl][:, dest] = arr.T
    return out, res


def kernel(**inputs) -> np.ndarray:
    out, _ = _run(inputs, trace=False)
    return out


# revision 29
# speedup vs baseline: 1.0227x; 1.0227x over previous
"""Trainium2 Bass kernel for nn_BayesianOddLayer (GNN message passing).

Computation (per reference):
    total_mask = w_odd2even_mask * odd_weights              # [E, E]
    z          = (u < sigmoid(dropout_logits))              # [E]
    msg        = x @ (total_mask * z[:, None])              # [B, E]
    skip       = llr @ (w_skipconn2even_mask * llr_weights) # [B, E]
    out        = tanh(0.5 * clip(msg + skip, -10, 10))

Structure exploited: w_odd2even_mask[e1, e2] is nonzero only when
var(e1) == var(e2) (Tanner graph), and the skip term feeds each edge
from exactly its own variable.  The 512 variables are packed into 16
OUTPUT TILES of exactly 128 edges each (whole variables per tile), and
the tiles' variables into 4 VAR-TILES of <= 128 variables.  Each output
tile is then TWO accumulating matmuls into the same PSUM region:
    skip: lhsT = masked lw block          [128 var-tile vars, 128 tile edges]
    msg : lhsT = masked/z-gated ow block  [128 tile edges, 128 tile edges]
The z-independent skip matmuls run FIRST (start=True) so the dropout
chain (u < sigmoid(logits), DVE+ACT) and the per-tile z-gating stay off
the ramp critical path; the msg matmuls accumulate behind them.

Engine budget per core: ACT tanh is the floor (16 groups x 2048
elems/lane at 1/cycle @1.2GHz + 352c pipeline fill ~ 32us) with matmuls
(~1.7us/group at 2.4GHz) hidden under it.  The combined DMA fabric
(~400 GB/s, and HALF that until the HAM clock gate releases ~4.5us
after first PE activity) binds the ramp, so the moving x operand and
the 0/1 masks ship as fp8 E3M4 (exact for masks; |x|<=5.4 < 15.5 max
normal, host-verified max output err 1.3e-2 vs the 2e-2 gate) while
weights/llr stay fp16 (fp8 weights would add another ~1.1e-2).  Mixed
fp16-lhsT x fp8-rhs matmuls are architecturally fine (both upcast to
fp22).  Output int8 (t*127 on DVE, |err| <= 0.004), halving stores.
Weight quads ride FOUR per-quad DMAs interleaved with the first rhs
group loads so quad q's prep completes just before group q's msg
matmuls; warmup matmuls bridge PE activity to the first real matmul
(an idle PE gap >~2us re-gates the clock to 1.2GHz).  Group 0's tanh
runs as two 1024-col halves so ACT starts ~1us earlier.  Stores:
chunks 0-1 as single 1MB SWDGE DMAs, chunk 2 on the by-then-idle sync
ring, last chunk per-group on alternating rings.

Sharding: data-parallel over batch across 8 NeuronCores; weights
replicated.
"""

from contextlib import ExitStack

import numpy as np
import ml_dtypes

import concourse.bass as bass
import concourse.mybir as mybir
from concourse import bacc
from concourse.bass_utils import run_bass_kernel_spmd
from concourse.tile import TileContext

F32 = mybir.dt.float32
F16 = mybir.dt.float16
F8 = mybir.dt.float8e3
I8 = mybir.dt.int8
U8 = mybir.dt.uint8
AF = mybir.ActivationFunctionType
ALU = mybir.AluOpType

E3M4 = ml_dtypes.float8_e3m4

B = 16384  # batch
E = 2048  # edges
NV = 512  # variable nodes
NCORES = 8
BSH = B // NCORES  # batch rows per core
CHUNK = 512  # batch columns per matmul (hw limit on the moving operand)
NCHUNK = BSH // CHUNK
P = 128  # partitions
NT = E // P  # output tiles (16), each exactly 128 edges
NQ = NT // 4  # quads = ACT groups per chunk (4)
NWARM = 12  # PE warmup matmuls (keep PE busy HAM-release -> first real matmul)
QSCALE = 127.0  # int8 output quantization scale

# byte-packed layouts (per partition row)
UHDB = 4 * NT * 2  # u/lg header bytes: NT f32 u + NT f32 lg = 128
WQB = 4 * P * 2 + 4 * P + 4 * P + 4 * P * 2  # quad: ew f16|em f8|sm f8|sw f16
EWO, EMO, SMO, SWO = 0, 4 * P * 2, 4 * P * 2 + 4 * P, 4 * P * 2 + 8 * P
GWB = CHUNK * 2 + 4 * CHUNK  # rhs (chunk, group) bytes: vt f16 | rt f8
W = NT * CHUNK  # out free-dim per chunk


def _plan_tiles(w_skipconn2even_mask: np.ndarray):
    """Pack whole variables into NT tiles of exactly P edges each, and the
    tiles' variables into NQ var-tiles of <= P variables (tile t's vars
    live in var-tile t//4).

    Returns (tile_edges [NT][P], vtile_vars [NQ][<=P]).
    """
    var = w_skipconn2even_mask.argmax(axis=0).astype(np.int64)  # [E]
    deg = np.bincount(var, minlength=NV)
    vars_nz = np.where(deg > 0)[0]
    order = vars_nz[np.argsort(-deg[vars_nz], kind="stable")]
    gsum = np.zeros(NT, np.int64)
    gnv = np.zeros(NT, np.int64)
    groups = [[] for _ in range(NT)]
    for v in order:
        dv = int(deg[v])
        cand = [g for g in range(NT) if gsum[g] + dv <= P]
        assert cand, "greedy packing failed"
        g = min(cand, key=lambda g: (gsum[g], gnv[g]))
        groups[g].append(int(v))
        gsum[g] += dv
        gnv[g] += 1
    assert all(s == P for s in gsum), f"imperfect packing {gsum}"

    # assign the 16 groups to 4 var-tiles (4 each), balancing #vars <= P
    tile_nv = np.zeros(NQ, np.int64)
    tile_cnt = np.zeros(NQ, np.int64)
    assign = [[] for _ in range(NQ)]
    for g in np.argsort(-gnv, kind="stable"):
        q = min(
            [q for q in range(NQ) if tile_cnt[q] < 4], key=lambda q: tile_nv[q]
        )
        assign[q].append(int(g))
        tile_nv[q] += gnv[g]
        tile_cnt[q] += 1
    assert all(n <= P for n in tile_nv), f"var-tile overflow {tile_nv}"

    edges_of = {v: np.where(var == v)[0] for v in vars_nz}
    tile_edges = []
    vtile_vars = []
    for q in range(NQ):
        vlist = []
        for g in assign[q]:
            gv = sorted(groups[g])
            vlist.extend(gv)
            te = np.concatenate([edges_of[v] for v in gv])
            assert te.size == P
            tile_edges.append(te)
        vtile_vars.append(np.array(vlist))
    assert sum(t.size for t in tile_edges) == E
    return tile_edges, vtile_vars


def _build_nc(need_clamp):
    nc = bacc.Bacc("TRN2", target_bir_lowering=False, debug=False,
                   num_devices=NCORES)
    wcomb = nc.dram_tensor(
        "wcomb", [P, UHDB + NQ * WQB], U8, kind="ExternalInput").ap()
    rhsp = nc.dram_tensor(
        "rhsp", [P, NCHUNK * NQ * GWB], U8, kind="ExternalInput").ap()
    outp = nc.dram_tensor("outp", [P, NCHUNK * W], I8, kind="ExternalOutput").ap()
    # the very last group stores fp16 directly (its int8 convert would sit
    # on the critical tail after the final tanh)
    outp16 = nc.dram_tensor("outp16", [P, 4 * CHUNK], F16, kind="ExternalOutput").ap()

    with TileContext(nc) as tc, ExitStack() as ctx:
        cpool = ctx.enter_context(tc.tile_pool(name="const", bufs=1))
        r0pool = ctx.enter_context(tc.tile_pool(name="rhs0", bufs=2))
        rcpool = ctx.enter_context(tc.tile_pool(name="rhsc", bufs=2))
        opool = ctx.enter_context(tc.tile_pool(name="out", bufs=4))
        o8pool = ctx.enter_context(tc.tile_pool(name="out8", bufs=4))
        pspool = ctx.enter_context(tc.tile_pool(name="ps", bufs=2, space="PSUM"))

        # PE warmup operands: memset first on gpsimd so warmups run during
        # the initial DMA window and keep the HAM clock gate open (HAM
        # re-evaluates every ~3.4us window on PE busy fraction)
        zl = cpool.tile([P, P], F16)
        nc.gpsimd.memset(zl[:], 0.0)
        zr = cpool.tile([P, CHUNK], F16)
        nc.gpsimd.memset(zr[:], 0.0)

        # u/lg rides its own tiny first DMA (small-transfer receipt fires
        # earliest); then quads 0-1 as ONE fat DMA (per-transfer fixed
        # latency ~2us makes many small DMAs strictly worse).  Quads 2-3
        # go right behind the first rhs load (separate tile: a shared tile
        # would add WAR hazards with quads 0-1 matmul reads).
        ut = cpool.tile([P, UHDB], U8)
        nc.sync.dma_start(ut[:], wcomb[:, 0:UHDB])
        wA = cpool.tile([P, 2 * WQB], U8)
        nc.sync.dma_start(wA[:, 0:WQB], wcomb[:, UHDB : UHDB + WQB])
        wB = cpool.tile([P, 2 * WQB], U8)

        def wq(q):
            t = wA if q < 2 else wB
            return t[:, (q % 2) * WQB : (q % 2 + 1) * WQB]

        def ew(q, i):  # [P tile edges, P out edges] f16
            w = wq(q)
            return w[:, EWO + i * 2 * P : EWO + (i + 1) * 2 * P].bitcast(F16)

        def em(q, i):  # 0/1 mask, fp8 (exact)
            w = wq(q)
            return w[:, EMO + i * P : EMO + (i + 1) * P].bitcast(F8)

        def sw(q, i):  # [P vars, P out edges] f16
            w = wq(q)
            return w[:, SWO + i * 2 * P : SWO + (i + 1) * 2 * P].bitcast(F16)

        # z = (u < sigmoid(dropout_logits)) in fp32 (u/logits arrive as raw
        # fp32 bytes; DVE copy feeds ACT a clean f32 tile since ACT cannot
        # take bitcast APs).  (Measured: a broadcast z-block tile + 512-col
        # TT prep, with or without high_priority pinning, reshuffles the
        # early schedule and LOSES 3-4us -- keep the per-tile STT form.)
        zt = cpool.tile([P, NT], F32)
        nc.vector.tensor_copy(zt[:], ut[:, 4 * NT : 8 * NT].bitcast(F32))
        nc.scalar.activation(zt[:], zt[:], AF.Sigmoid)
        nc.vector.tensor_tensor(
            zt[:], ut[:, 0 : 4 * NT].bitcast(F32), zt[:], ALU.is_lt)

        wps = pspool.tile([P, 4 * CHUNK], F32, tag="ps")
        for _ in range(NWARM):
            nc.tensor.matmul(wps[:, 0:CHUNK], zl[:], zr[:], start=True, stop=True)

        # weight prep (all DVE; gpsimd elementwise is 35x slower): skip
        # block first (sw *= sm, one 512-col op, unblocks the z-independent
        # skip matmuls), then (ew * z[tile]) * em fused per tile
        def prep_quad(q):
            w = wq(q)
            swb = w[:, SWO : SWO + 8 * P].bitcast(F16)
            smb = w[:, SMO : SMO + 4 * P].bitcast(F8)
            nc.vector.tensor_tensor(swb, swb, smb, ALU.mult)
            for i in range(4):
                t = 4 * q + i
                nc.vector.scalar_tensor_tensor(
                    ew(q, i), ew(q, i), zt[:, t : t + 1], em(q, i),
                    ALU.mult, ALU.mult)

        prep_quad(0)

        for nb in range(NCHUNK):
            for q in range(NQ):
                # rhs loads: chunks 0-1 as per-group DMAs with the quad-2/3
                # weight transfers interleaved so every group's operands
                # land just before its matmuls (these all transfer after
                # the HAM clock release, where per-transfer overhead is
                # small); chunks 2-3 as single whole-chunk DMAs into
                # separate buffers (no WAR gating, issued back to back)
                if nb <= 1:
                    if nb == 0 and q == 0:
                        # (measured: moving these to the gpsimd ring to run
                        # both rings during the ramp THROTTLES the fabric,
                        # +1.0-1.5us stalls at T2-T4 -- keep one ring)
                        r01 = []
                        for g in range(2 * NQ):
                            rg = r0pool.tile([P, GWB], U8, name=f"rg{g}",
                                             bufs=1)
                            r01.append(rg)
                            nc.sync.dma_start(
                                rg[:], rhsp[:, g * GWB : (g + 1) * GWB])
                            if g == 0:
                                nc.sync.dma_start(
                                    wA[:, WQB:],
                                    wcomb[:, UHDB + WQB : UHDB + 2 * WQB])
                                prep_quad(1)
                            if g == 1:
                                nc.sync.dma_start(
                                    wB[:, 0:WQB],
                                    wcomb[:, UHDB + 2 * WQB : UHDB + 3 * WQB])
                                prep_quad(2)
                            if g == 2:
                                nc.sync.dma_start(
                                    wB[:, WQB:],
                                    wcomb[:, UHDB + 3 * WQB : UHDB + 4 * WQB])
                                prep_quad(3)
                    rtv = r01[nb * NQ + q][:]
                else:
                    if q == 0:
                        rc = rcpool.tile([P, NQ * GWB], U8, name="rc")
                        c0 = nb * NQ * GWB
                        nc.sync.dma_start(rc[:], rhsp[:, c0 : c0 + NQ * GWB])
                    rtv = rc[:, q * GWB : (q + 1) * GWB]
                vt = rtv[:, 0 : 2 * CHUNK].bitcast(F16)

                def rt(i):
                    o = 2 * CHUNK + i * CHUNK
                    return rtv[:, o : o + CHUNK].bitcast(F8)

                ps = pspool.tile([P, 4 * CHUNK], F32)
                # z-independent skip matmuls first, then the accumulating
                # z-gated msg matmuls.  Group (0,0) interleaves per HALF
                # (skip01, msg01, skip23, msg23 -- no same-bank neighbors)
                # so its first tanh half fires after 4 matmuls, ~0.9us
                # earlier.
                halves = [(0, 1), (2, 3)] if nb == 0 and q == 0 else [(0, 1, 2, 3)]
                for idx in halves:
                    for i in idx:
                        nc.tensor.matmul(
                            ps[:, i * CHUNK : (i + 1) * CHUNK],
                            sw(q, i), vt, start=True, stop=False)
                    for i in idx:
                        nc.tensor.matmul(
                            ps[:, i * CHUNK : (i + 1) * CHUNK],
                            ew(q, i), rt(i), start=False, stop=True)
                last = nb == NCHUNK - 1 and q == NQ - 1
                ot = opool.tile([P, 4 * CHUNK], F16)
                if need_clamp:
                    nc.vector.tensor_scalar(
                        ot[:], ps[:], 10.0, -10.0, ALU.min, ALU.max)
                    nc.scalar.activation(ot[:], ot[:], AF.Tanh, scale=0.5)
                elif nb == 0 and q == 0:
                    # first group in two halves: ACT starts ~1us earlier
                    h = 2 * CHUNK
                    nc.scalar.activation(
                        ot[:, 0:h], ps[:, 0:h], AF.Tanh, scale=0.5)
                    nc.scalar.activation(
                        ot[:, h:], ps[:, h:], AF.Tanh, scale=0.5)
                elif last:
                    # final group in two halves: the first half's store
                    # drains (gpsimd ring, so it never queues ahead of the
                    # very last sync-ring store) while the second half's
                    # tanh still runs
                    h = 2 * CHUNK
                    nc.scalar.activation(
                        ot[:, 0:h], ps[:, 0:h], AF.Tanh, scale=0.5)
                    nc.gpsimd.dma_start(outp16[:, 0:h], ot[:, 0:h])
                    nc.scalar.activation(
                        ot[:, h:], ps[:, h:], AF.Tanh, scale=0.5)
                else:
                    # clip(v, +-10) proven identity for these inputs (host
                    # bound); tanh straight from PSUM
                    nc.scalar.activation(ot[:], ps[:], AF.Tanh, scale=0.5)
                # int8 quantize on DVE (round-to-nearest, ~1.2us per group)
                if nb < NCHUNK - 2:
                    # chunks 0-1: quantize into a per-chunk tile, store as
                    # ONE 1MB DMA (small SWDGE stores sustain only ~110GB/s
                    # under load contention, and these overlap the loads)
                    if q == 0:
                        o8c = o8pool.tile([P, NT * CHUNK], I8, tag="o8c", bufs=2)
                    nc.vector.tensor_scalar(
                        o8c[:, q * 4 * CHUNK : (q + 1) * 4 * CHUNK],
                        ot[:], QSCALE, None, ALU.mult)
                    if q == NQ - 1:
                        nc.gpsimd.dma_start(outp[:, nb * W : (nb + 1) * W], o8c[:])
                elif nb == NCHUNK - 2:
                    # chunk 2: loads are done by now -- store per-group on
                    # the gpsimd ring so the drain spreads out instead of
                    # bunching a 1MB store into the tail
                    o8 = o8pool.tile([P, 4 * CHUNK], I8)
                    nc.vector.tensor_scalar(o8[:], ot[:], QSCALE, None, ALU.mult)
                    c0 = nb * W + q * 4 * CHUNK
                    nc.gpsimd.dma_start(outp[:, c0 : c0 + 4 * CHUNK], o8[:])
                elif q < NQ - 1:
                    # last chunk: per-group stores alternating rings so the
                    # final drain is parallel and fine-grained
                    o8 = o8pool.tile([P, 4 * CHUNK], I8)
                    nc.vector.tensor_scalar(o8[:], ot[:], QSCALE, None, ALU.mult)
                    c0 = nb * W + q * 4 * CHUNK
                    if q % 2 == 0:
                        nc.sync.dma_start(outp[:, c0 : c0 + 4 * CHUNK], o8[:])
                    else:
                        nc.gpsimd.dma_start(outp[:, c0 : c0 + 4 * CHUNK], o8[:])
                elif need_clamp:
                    nc.sync.dma_start(outp16[:], ot[:])
                else:
                    # final group: fp16 straight out, no convert on the
                    # tail (first half's store was issued above); the very
                    # last store is split across BOTH rings -- per-transfer
                    # latency (~2us) dominates 256KB stores, so two
                    # parallel 128KB transfers finish ~1us sooner
                    nc.sync.dma_start(
                        outp16[:, 2 * CHUNK : 3 * CHUNK],
                        ot[:, 2 * CHUNK : 3 * CHUNK])
                    nc.gpsimd.dma_start(
                        outp16[:, 3 * CHUNK :], ot[:, 3 * CHUNK :])
    nc.compile()
    return nc


def _prep(x, llr, u, odd_weights, llr_weights, dropout_logits,
          w_odd2even_mask, w_skipconn2even_mask):
    """Host-side data movement: tile packing, block gathers, shards, casts."""
    ow = np.asarray(odd_weights, np.float32)
    msk = np.asarray(w_odd2even_mask, np.float32)
    lw = np.asarray(llr_weights, np.float32)
    smask = np.asarray(w_skipconn2even_mask, np.float32)
    u = np.asarray(u, np.float32)
    lg = np.asarray(dropout_logits, np.float32)

    tile_edges, vtile_vars = _plan_tiles(smask)

    wblk = np.zeros((P, NQ * WQB), np.uint8)
    ucomb = np.zeros((P, NT), np.float32)
    lgcomb = np.zeros((P, NT), np.float32)
    for t in range(NT):
        q = t // 4
        i = t % 4
        pe = tile_edges[t]
        vs = vtile_vars[q]
        c = q * WQB
        wblk[:, c + EWO + i * 2 * P : c + EWO + (i + 1) * 2 * P] = (
            ow[np.ix_(pe, pe)].astype(np.float16).view(np.uint8))
        wblk[:, c + EMO + i * P : c + EMO + (i + 1) * P] = (
            msk[np.ix_(pe, pe)].astype(E3M4).view(np.uint8))
        wblk[: vs.size, c + SWO + i * 2 * P : c + SWO + (i + 1) * 2 * P] = (
            lw[np.ix_(vs, pe)].astype(np.float16).view(np.uint8))
        wblk[: vs.size, c + SMO + i * P : c + SMO + (i + 1) * P] = (
            smask[np.ix_(vs, pe)].astype(E3M4).view(np.uint8))
        ucomb[:, t] = u[pe]
        lgcomb[:, t] = lg[pe]

    x = np.asarray(x, np.float32)
    llr = np.asarray(llr, np.float32)

    # Rigorous bound on |msg + skip| with the exact shipped (quantized)
    # values: if it cannot reach the +-10 clip, the clip is the identity
    # and the device clamp stage is elided.
    xmax = float(np.abs(x.astype(E3M4).astype(np.float32)).max())
    lmax = float(np.abs(llr.astype(np.float16).astype(np.float32)).max())
    awe = np.zeros((NT, P), np.float64)  # sum_e' |ow*mask| per out edge
    aws = np.zeros((NT, P), np.float64)
    for t in range(NT):
        q = t // 4
        c = q * WQB
        ewf = wblk[:, c + EWO + t % 4 * 2 * P : c + EWO + (t % 4 + 1) * 2 * P
                   ].view(np.float16).astype(np.float64)
        emf = wblk[:, c + EMO + t % 4 * P : c + EMO + (t % 4 + 1) * P
                   ].view(E3M4).astype(np.float64)
        swf = wblk[:, c + SWO + t % 4 * 2 * P : c + SWO + (t % 4 + 1) * 2 * P
                   ].view(np.float16).astype(np.float64)
        smf = wblk[:, c + SMO + t % 4 * P : c + SMO + (t % 4 + 1) * P
                   ].view(E3M4).astype(np.float64)
        awe[t] = np.abs(ewf * emf).sum(axis=0)
        aws[t] = np.abs(swf * smf).sum(axis=0)
    bound = float((awe * xmax + aws * lmax).max())
    need_clamp = bound >= 9.5

    in_maps = []
    for c in range(NCORES):
        sl = slice(c * BSH, (c + 1) * BSH)
        xq = np.ascontiguousarray(x[sl].T).astype(E3M4).view(np.uint8)  # [E, BSH]
        lq = np.ascontiguousarray(llr[sl].T).astype(np.float16)  # [NV, BSH]
        rhsp = np.zeros((P, NCHUNK, NQ, GWB), np.uint8)
        for q in range(NQ):
            vs = vtile_vars[q]
            vtb = lq[vs].view(np.uint8).reshape(vs.size, NCHUNK, 2 * CHUNK)
            rhsp[: vs.size, :, q, 0 : 2 * CHUNK] = vtb.transpose(0, 1, 2)
            for i in range(4):
                t = 4 * q + i
                rtb = xq[tile_edges[t]].reshape(P, NCHUNK, CHUNK)
                o = 2 * CHUNK + i * CHUNK
                rhsp[:, :, q, o : o + CHUNK] = rtb
        in_maps.append({
            "wcomb": np.ascontiguousarray(np.concatenate(
                [ucomb.view(np.uint8), lgcomb.view(np.uint8), wblk], axis=1)),
            "rhsp": np.ascontiguousarray(rhsp.reshape(P, -1)),
        })
    return tile_edges, in_maps, need_clamp


def _run(inputs: dict, trace: bool = False, **kwargs):
    tile_edges, in_maps, need_clamp = _prep(**inputs)
    nc = _build_nc(need_clamp)
    res = run_bass_kernel_spmd(nc, in_maps, list(range(NCORES)), trace=trace, **kwargs)

    dest = np.concatenate(tile_edges)  # row (t, p) -> edge column
    out = np.empty((B, E), np.float32)
    for c in range(NCORES):
        sl = slice(c * BSH, (c + 1) * BSH)
        a8 = (res.results[c]["outp"]
              .reshape(P, NCHUNK, NT, CHUNK)
              .astype(np.float32) * np.float32(1.0 / QSCALE))
        # last group (chunk 3, tiles 12-15) arrived as raw fp16
        a16 = (res.results[c]["outp16"]
               .reshape(P, 4, CHUNK)
               .astype(np.float32))
        a8[:, NCHUNK - 1, NT - 4 : NT, :] = a16
        arr = a8.transpose(2, 0, 1, 3).reshape(NT * P, BSH)
        out[sl][:, dest] = arr.T
    return out, res


def kernel(**inputs) -> np.ndarray:
    out, _ = _run(inputs, trace=False)
    return out
